# revision 1
# baseline (speedup 1.0000x reference)
"""Switch-Transformer top-1 MoE forward on 8 Trainium2 NeuronCores.

Strategy (expert-parallel, per the sharding hint):
  - Router (logits/softmax/argmax/positions) + the aux/z losses run on host
    via jax-on-CPU, mirroring the reference op-for-op so routing decisions
    match it bit-exactly.
  - Dispatch scatters tokens into per-expert buffers [E, cap, D] on host and
    hands expert e's buffer (transposed to [D, cap]) to core e.
  - Each core runs its expert's FFN out = relu(buf@W1+b1)@W2 + b2 as one Bass
    kernel, entirely in transposed layout so no on-device transposes:
      GEMM1: hT[f, c]  = sum_d W1[d, f] * bufT[d, c]   (lhsT = W1 tile)
      GEMM2: obT[d, c] = sum_f W2[f, d] * hT[f, c]     (lhsT = W2 tile)
    Matmuls run in float32r (full PE rate, ~1.7e-4 rel err end to end).
  - Combine gathers each token's column from its expert's output and applies
    the gate on host.

Weights are pre-swizzled on host so every device DMA is contiguous:
  w1s[m, ki, ko, f] = W1[ko*128+ki, m*128+f]   m in 0..32 (F tiles)
  w2s[m, ki, ko, d] = W2[ko*128+ki, m*128+d]   m in 0..8  (D tiles)
"""

import os
import sys

import numpy as np

for _p in ("/opt/trn_rl_repo", "/root/.axon_site/_ro/trn_rl_repo"):
    if os.path.isdir(_p) and _p not in sys.path:
        sys.path.append(_p)

import concourse.bass as bass
import concourse.mybir as mybir
import concourse.tile as tile
from concourse.bass import ds
from concourse.bass_utils import run_bass_kernel_spmd
from concourse.vector_clock import ScopedClock

# ---------------------------------------------------------------------------
# TileContext workaround: this walrus build accepts at most ONE sync-wait per
# instruction ("Too many sync wait commands" in setupSyncWait otherwise).
# Split any multi-wait instruction into preceding single-wait NOPs, and chunk
# the kernel-tail drain's global-clock waits the same way.
# ---------------------------------------------------------------------------


class PatchedTileContext(tile.TileContext):
    def _add_instruction(self, inst):
        si = inst.sync_info
        if (
            si is not None
            and len(si.on_wait) > 1
            and inst.engine != mybir.EngineType.Unassigned
        ):
            waits = list(si.on_wait)
            for w in waits[:-1]:
                nop = mybir.InstNoOp(
                    name=self.nc.get_next_instruction_name(),
                    engine=inst.engine,
                    sync_info=mybir.SyncInfo(on_wait=[w], on_update=[]),
                    bass_nofuse=True,
                )
                super()._add_instruction(nop)
            inst.sync_info = mybir.SyncInfo(
                on_wait=[waits[-1]], on_update=list(si.on_update)
            )
        super()._add_instruction(inst)

    def _drain_and_barrier(self, tick_clock, wait_clock):
        nc = self.nc
        dummy = mybir.InstNoOp(
            name=nc.get_next_instruction_name(),
            bass_nofuse=True,
            engine=mybir.EngineType.SP,
        )
        wait_clock.add_sem_waits(dummy, ScopedClock({None: tick_clock.global_clock}))
        waits = list(dummy.sync_info.on_wait) if dummy.sync_info is not None else []
        for w in waits:
            nc.sync.add_instruction(
                mybir.InstNoOp(
                    name=nc.get_next_instruction_name(),
                    sync_info=mybir.SyncInfo(on_wait=[w], on_update=[]),
                    bass_nofuse=True,
                    engine=mybir.EngineType.SP,
                )
            )
        nc.sync.drain()
        nc.all_engine_barrier()
        assert self.sems is not None
        popped = nc._tile_sem_poison_stack.pop()
        assert popped is self._sem_poison
        nc.clear_and_free_semaphores(list(self.sems.allocated().values()))
        nc.all_engine_barrier()


# ---------------------------------------------------------------------------
# Problem shapes (hardcoded per the contract)
# ---------------------------------------------------------------------------

B, S, D, E, F = 2, 2048, 1024, 8, 4096
T = B * S
CAP = int(2.0 * T / E)  # 1024 per-expert capacity
C = CAP

P = 128
KO1, MT1 = D // P, F // P  # GEMM1: 8 k-tiles, 32 m-tiles
KO2, MT2 = F // P, D // P  # GEMM2: 32 k-tiles, 8 m-tiles
N = 512                    # PSUM free dim per matmul
NT = C // N
WCH = 8                    # k-tiles per streamed weight chunk

F32 = mybir.dt.float32
F32R = mybir.dt.float32r


def build_expert_ffn():
    nc = bass.Bass("TRN2", target_bir_lowering=False, debug=False, num_devices=8)

    bufT = nc.declare_dram_parameter("bufT", [D, C], F32R, isOutput=False)
    w1s = nc.declare_dram_parameter("w1s", [MT1, P, KO1, P], F32R, isOutput=False)
    w2s = nc.declare_dram_parameter("w2s", [MT2, P, KO2, P], F32R, isOutput=False)
    b1c = nc.declare_dram_parameter("b1c", [P, MT1], F32, isOutput=False)
    b2c = nc.declare_dram_parameter("b2c", [P, MT2], F32, isOutput=False)
    out = nc.declare_dram_parameter("out", [MT2, P, C], F32, isOutput=True)

    with PatchedTileContext(nc) as tc:
        with (
            tc.tile_pool(name="const", bufs=1) as const_pool,
            tc.tile_pool(name="hbuf", bufs=1) as h_pool,
            tc.tile_pool(name="wstream", bufs=6) as w_pool,
            tc.tile_pool(name="obuf", bufs=2) as o_pool,
            tc.tile_pool(name="psA", bufs=4, space="PSUM") as psA,
            tc.tile_pool(name="psB", bufs=4, space="PSUM") as psB,
        ):
            bufT_v = bufT.ap().rearrange("(ko ki) c -> ki ko c", ki=P)
            # one tile per k-chunk so the first matmuls start after ~1/8 of
            # the input load instead of waiting on the whole tensor
            xts = [
                const_pool.tile([P, C], F32R, tag=f"xt{ko}", name=f"xt_{ko}")
                for ko in range(KO1)
            ]
            b1t = const_pool.tile([P, MT1], F32, tag="b1t")
            b2t = const_pool.tile([P, MT2], F32, tag="b2t")

            ht = h_pool.tile([P, KO2, C], F32R, tag="ht")

            def relu_drain(pt, m, n):
                # relu(psum + b1[:, m]) -> hT[:, m, n*N:...]
                nc.vector.tensor_scalar(
                    out=ht[:, m, ds(n * N, N)],
                    in0=pt[:],
                    scalar1=b1t[:, ds(m, 1)],
                    scalar2=0.0,
                    op0=mybir.AluOpType.add,
                    op1=mybir.AluOpType.max,
                )

            # ---- Phase A: hT = relu(W1.T @ bufT + b1) ----
            # Startup block m=0..3 runs k-outer across 8 concurrent PSUM
            # groups (borrowing phase B's banks, idle this early) so PE work
            # overlaps the interleaved input-chunk loads.
            SB = 4
            w1ts = []
            for m in range(SB):
                w1t = w_pool.tile([P, WCH, P], F32R, tag="w", name=f"w1t_{m}")
                nc.sync.dma_start(w1t[:], w1s.ap()[m])
                w1ts.append(w1t)
                if m < len(xts):
                    nc.sync.dma_start(xts[m][:], bufT_v[:, m, :])
            nc.sync.dma_start(b1t[:], b1c.ap())
            nc.sync.dma_start(b2t[:], b2c.ap())
            for ko in range(SB, KO1):
                nc.sync.dma_start(xts[ko][:], bufT_v[:, ko, :])

            pgrp = [
                [
                    (psA if m < 2 else psB).tile(
                        [P, N],
                        F32,
                        tag=("psA" if m < 2 else "psB"),
                        name=f"psS_{m}_{n}",
                    )
                    for n in range(NT)
                ]
                for m in range(SB)
            ]
            for k in range(KO1):
                for m in range(SB):
                    for n in range(NT):
                        nc.tensor.matmul(
                            pgrp[m][n][:],
                            w1ts[m][:, k, :],
                            xts[k][:, ds(n * N, N)],
                            start=(k == 0),
                            stop=(k == KO1 - 1),
                        )
            for m in range(SB):
                for n in range(NT):
                    relu_drain(pgrp[m][n], m, n)

            for m in range(SB, MT1):
                w1t = w_pool.tile([P, WCH, P], F32R, tag="w", name=f"w1t_{m}")
                nc.sync.dma_start(w1t[:], w1s.ap()[m])
                for n in range(NT):
                    pt = psA.tile([P, N], F32, tag="psA", name=f"psA_{m}_{n}")
                    for k in range(KO1):
                        nc.tensor.matmul(
                            pt[:],
                            w1t[:, k, :],
                            xts[k][:, ds(n * N, N)],
                            start=(k == 0),
                            stop=(k == KO1 - 1),
                        )
                    relu_drain(pt, m, n)

            # ---- Phase B: obT = W2.T @ hT + b2 ----
            for m in range(MT2):
                ot = o_pool.tile([P, C], F32, tag="ot", name=f"ot_{m}")
                pts = [
                    psB.tile([P, N], F32, tag="psB", name=f"psB_{m}_{n}")
                    for n in range(NT)
                ]
                for c in range(KO2 // WCH):
                    w2t = w_pool.tile(
                        [P, WCH, P], F32R, tag="w", name=f"w2t_{m}_{c}"
                    )
                    nc.sync.dma_start(w2t[:], w2s.ap()[m, :, ds(c * WCH, WCH), :])
                    for k in range(WCH):
                        k2 = c * WCH + k
                        for n in range(NT):
                            nc.tensor.matmul(
                                pts[n][:],
                                w2t[:, k, :],
                                ht[:, k2, ds(n * N, N)],
                                start=(k2 == 0),
                                stop=(k2 == KO2 - 1),
                            )
                for n in range(NT):
                    nc.vector.tensor_scalar_add(
                        out=ot[:, ds(n * N, N)],
                        in0=pts[n][:],
                        scalar1=b2t[:, ds(m, 1)],
                    )
                    nc.sync.dma_start(
                        out.ap()[m, :, ds(n * N, N)], ot[:, ds(n * N, N)]
                    )

    return nc


_NC_CACHE: list = []


def _get_nc():
    if not _NC_CACHE:
        _NC_CACHE.append(build_expert_ffn())
    return _NC_CACHE[0]


# ---------------------------------------------------------------------------
# Host-side routing / dispatch / combine
# ---------------------------------------------------------------------------


def _router_host(x, Wg):
    """Top-1 routing + losses, mirroring the reference op-for-op on jax CPU so
    routing decisions match it bit-exactly."""
    import jax
    import jax.numpy as jnp

    cpu = jax.devices("cpu")[0]
    with jax.default_device(cpu):
        xf = jnp.asarray(np.asarray(x, dtype=np.float32)).reshape(T, D)
        logits = xf @ jnp.asarray(np.asarray(Wg, dtype=np.float32))
        probs = jax.nn.softmax(logits, axis=-1)
        eidx = jnp.argmax(probs, axis=-1)
        gate = jnp.take_along_axis(probs, eidx[:, None], axis=1)[:, 0]
        oh = jax.nn.one_hot(eidx, E, dtype=jnp.int32)
        pos = jnp.cumsum(oh, axis=0)[jnp.arange(T), eidx] - 1
        keep = pos < CAP
        posc = jnp.clip(pos, 0, CAP - 1)
        frac = jnp.mean(oh.astype(xf.dtype), axis=0)
        pmean = jnp.mean(probs, axis=0)
        aux_loss = E * jnp.sum(frac * pmean)
        z_loss = jnp.mean(jax.scipy.special.logsumexp(logits, axis=-1) ** 2)
        return (
            np.asarray(eidx),
            np.asarray(gate),
            np.asarray(keep),
            np.asarray(posc),
            np.asarray(aux_loss),
            np.asarray(z_loss),
        )


def kernel(x, Wg, W1, b1, W2, b2):
    x = np.asarray(x, dtype=np.float32)
    Wg = np.asarray(Wg, dtype=np.float32)
    W1 = np.asarray(W1, dtype=np.float32)
    b1 = np.asarray(b1, dtype=np.float32)
    W2 = np.asarray(W2, dtype=np.float32)
    b2 = np.asarray(b2, dtype=np.float32)

    eidx, gate, keep, posc, aux_loss, z_loss = _router_host(x, Wg)

    # dispatch: scatter kept tokens into per-expert buffers, transposed
    xf = np.ascontiguousarray(x.reshape(T, D))
    buf = np.zeros((E, CAP, D), dtype=np.float32)
    buf[eidx[keep], posc[keep]] = xf[keep]
    bufT = np.ascontiguousarray(buf.transpose(0, 2, 1))  # [E, D, CAP]

    # weight swizzles -> contiguous per-tile DMA layouts
    w1s = np.ascontiguousarray(
        W1.reshape(E, KO1, P, MT1, P).transpose(0, 3, 2, 1, 4)
    )
    w2s = np.ascontiguousarray(
        W2.reshape(E, KO2, P, MT2, P).transpose(0, 3, 2, 1, 4)
    )
    b1c = np.ascontiguousarray(b1.reshape(E, MT1, P).transpose(0, 2, 1))
    b2c = np.ascontiguousarray(b2.reshape(E, MT2, P).transpose(0, 2, 1))

    in_maps = [
        {
            "bufT": bufT[e],
            "w1s": w1s[e],
            "w2s": w2s[e],
            "b1c": b1c[e],
            "b2c": b2c[e],
        }
        for e in range(E)
    ]
    res = run_bass_kernel_spmd(_get_nc(), in_maps, list(range(E)), trace=False)

    # combine: ob_all[e] is obT = [D, CAP]; token t reads column posc[t]
    ob_all = np.stack([res.results[e]["out"].reshape(D, CAP) for e in range(E)])
    y = ob_all[eidx, :, posc] * (gate * keep.astype(np.float32))[:, None]
    y = np.ascontiguousarray(y.reshape(B, S, D).astype(np.float32))

    return y, np.float32(aux_loss), np.float32(z_loss)


# revision 2
# speedup vs baseline: 1.2378x; 1.2378x over previous
"""Switch-Transformer top-1 MoE forward on 8 Trainium2 NeuronCores.

Strategy (expert-parallel, per the sharding hint):
  - Router (logits/softmax/argmax/positions) + the aux/z losses run on host
    via jax-on-CPU, mirroring the reference op-for-op so routing decisions
    match it bit-exactly.
  - Dispatch scatters tokens into per-expert buffers [E, cap, D] on host and
    hands expert e's buffer (transposed to [D, cap]) to core e.
  - Each core runs its expert's FFN out = relu(buf@W1+b1)@W2 + b2 as one Bass
    kernel, entirely in transposed layout so no on-device transposes:
      GEMM1: hT[f, c]  = sum_d W1[d, f] * bufT[d, c]   (lhsT = W1 tile)
      GEMM2: obT[d, c] = sum_f W2[f, d] * hT[f, c]     (lhsT = W2 tile)
    Matmuls run in float32r (full PE rate, ~1.7e-4 rel err end to end).
  - Combine gathers each token's column from its expert's output and applies
    the gate on host.

Weights are pre-swizzled on host so every device DMA is contiguous:
  w1s[m, ki, ko, f] = W1[ko*128+ki, m*128+f]   m in 0..32 (F tiles)
  w2s[m, ki, ko, d] = W2[ko*128+ki, m*128+d]   m in 0..8  (D tiles)
"""

import os
import sys

import numpy as np

for _p in ("/opt/trn_rl_repo", "/root/.axon_site/_ro/trn_rl_repo"):
    if os.path.isdir(_p) and _p not in sys.path:
        sys.path.append(_p)

import concourse.bass as bass
import concourse.mybir as mybir
import concourse.tile as tile
from concourse.bass import ds
from concourse.bass_utils import run_bass_kernel_spmd
from concourse.vector_clock import ScopedClock

# ---------------------------------------------------------------------------
# TileContext workaround: this walrus build accepts at most ONE sync-wait per
# instruction ("Too many sync wait commands" in setupSyncWait otherwise).
# Split any multi-wait instruction into preceding single-wait NOPs, and chunk
# the kernel-tail drain's global-clock waits the same way.
# ---------------------------------------------------------------------------


class PatchedTileContext(tile.TileContext):
    def _add_instruction(self, inst):
        si = inst.sync_info
        if (
            si is not None
            and len(si.on_wait) > 1
            and inst.engine != mybir.EngineType.Unassigned
        ):
            waits = list(si.on_wait)
            for w in waits[:-1]:
                nop = mybir.InstNoOp(
                    name=self.nc.get_next_instruction_name(),
                    engine=inst.engine,
                    sync_info=mybir.SyncInfo(on_wait=[w], on_update=[]),
                    bass_nofuse=True,
                )
                super()._add_instruction(nop)
            inst.sync_info = mybir.SyncInfo(
                on_wait=[waits[-1]], on_update=list(si.on_update)
            )
        super()._add_instruction(inst)

    def _drain_and_barrier(self, tick_clock, wait_clock):
        nc = self.nc
        dummy = mybir.InstNoOp(
            name=nc.get_next_instruction_name(),
            bass_nofuse=True,
            engine=mybir.EngineType.SP,
        )
        wait_clock.add_sem_waits(dummy, ScopedClock({None: tick_clock.global_clock}))
        waits = list(dummy.sync_info.on_wait) if dummy.sync_info is not None else []
        for w in waits:
            nc.sync.add_instruction(
                mybir.InstNoOp(
                    name=nc.get_next_instruction_name(),
                    sync_info=mybir.SyncInfo(on_wait=[w], on_update=[]),
                    bass_nofuse=True,
                    engine=mybir.EngineType.SP,
                )
            )
        nc.sync.drain()
        nc.all_engine_barrier()
        assert self.sems is not None
        popped = nc._tile_sem_poison_stack.pop()
        assert popped is self._sem_poison
        nc.clear_and_free_semaphores(list(self.sems.allocated().values()))
        nc.all_engine_barrier()


# ---------------------------------------------------------------------------
# Problem shapes (hardcoded per the contract)
# ---------------------------------------------------------------------------

B, S, D, E, F = 2, 2048, 1024, 8, 4096
T = B * S
CAP = int(2.0 * T / E)  # 1024 per-expert capacity
C = CAP

P = 128
KO1, MT1 = D // P, F // P  # GEMM1: 8 k-tiles, 32 m-tiles
KO2, MT2 = F // P, D // P  # GEMM2: 32 k-tiles, 8 m-tiles
N = 512                    # PSUM free dim per matmul
NT = C // N
WCH = 8                    # k-tiles per streamed weight chunk

F32 = mybir.dt.float32
F32R = mybir.dt.float32r


def build_expert_ffn():
    nc = bass.Bass("TRN2", target_bir_lowering=False, debug=False, num_devices=8)

    bufT = nc.declare_dram_parameter("bufT", [D, C], F32R, isOutput=False)
    w1s = nc.declare_dram_parameter("w1s", [MT1, P, KO1, P], F32R, isOutput=False)
    w2s = nc.declare_dram_parameter("w2s", [MT2, P, KO2, P], F32R, isOutput=False)
    b1c = nc.declare_dram_parameter("b1c", [P, MT1], F32, isOutput=False)
    b2c = nc.declare_dram_parameter("b2c", [P, MT2], F32, isOutput=False)
    out = nc.declare_dram_parameter("out", [MT2, P, C], F32, isOutput=True)

    with PatchedTileContext(nc) as tc:
        with (
            tc.tile_pool(name="const", bufs=1) as const_pool,
            tc.tile_pool(name="hbuf", bufs=1) as h_pool,
            tc.tile_pool(name="wstream", bufs=6) as w_pool,
            tc.tile_pool(name="obuf", bufs=2) as o_pool,
            tc.tile_pool(name="psA", bufs=4, space="PSUM") as psA,
            tc.tile_pool(name="psB", bufs=4, space="PSUM") as psB,
        ):
            bufT_v = bufT.ap().rearrange("(ko ki) c -> ki ko c", ki=P)
            # one tile per k-chunk so the first matmuls start after ~1/8 of
            # the input load instead of waiting on the whole tensor
            xts = [
                const_pool.tile([P, C], F32R, tag=f"xt{ko}", name=f"xt_{ko}")
                for ko in range(KO1)
            ]
            b1t = const_pool.tile([P, MT1], F32, tag="b1t")
            b2t = const_pool.tile([P, MT2], F32, tag="b2t")

            ht = h_pool.tile([P, KO2, C], F32R, tag="ht")

            def relu_drain(pt, m, n):
                # relu(psum + b1[:, m]) -> hT[:, m, n*N:...]
                nc.vector.tensor_scalar(
                    out=ht[:, m, ds(n * N, N)],
                    in0=pt[:],
                    scalar1=b1t[:, ds(m, 1)],
                    scalar2=0.0,
                    op0=mybir.AluOpType.add,
                    op1=mybir.AluOpType.max,
                )

            # ---- Phase A: hT = relu(W1.T @ bufT + b1) ----
            # Startup block m=0..3 runs k-outer across 8 concurrent PSUM
            # groups (borrowing phase B's banks, idle this early) so PE work
            # overlaps the interleaved input-chunk loads.
            SB = 4
            w1ts = []
            for m in range(SB):
                w1t = w_pool.tile([P, WCH, P], F32R, tag="w", name=f"w1t_{m}")
                nc.sync.dma_start(w1t[:], w1s.ap()[m])
                w1ts.append(w1t)
                if m < len(xts):
                    nc.sync.dma_start(xts[m][:], bufT_v[:, m, :])
            nc.sync.dma_start(b1t[:], b1c.ap())
            nc.sync.dma_start(b2t[:], b2c.ap())
            for ko in range(SB, KO1):
                nc.sync.dma_start(xts[ko][:], bufT_v[:, ko, :])

            pgrp = [
                [
                    (psA if m < 2 else psB).tile(
                        [P, N],
                        F32,
                        tag=("psA" if m < 2 else "psB"),
                        name=f"psS_{m}_{n}",
                    )
                    for n in range(NT)
                ]
                for m in range(SB)
            ]
            for k in range(KO1):
                for m in range(SB):
                    for n in range(NT):
                        nc.tensor.matmul(
                            pgrp[m][n][:],
                            w1ts[m][:, k, :],
                            xts[k][:, ds(n * N, N)],
                            start=(k == 0),
                            stop=(k == KO1 - 1),
                        )
            for m in range(SB):
                for n in range(NT):
                    relu_drain(pgrp[m][n], m, n)

            for m in range(SB, MT1):
                w1t = w_pool.tile([P, WCH, P], F32R, tag="w", name=f"w1t_{m}")
                nc.sync.dma_start(w1t[:], w1s.ap()[m])
                for n in range(NT):
                    pt = psA.tile([P, N], F32, tag="psA", name=f"psA_{m}_{n}")
                    for k in range(KO1):
                        nc.tensor.matmul(
                            pt[:],
                            w1t[:, k, :],
                            xts[k][:, ds(n * N, N)],
                            start=(k == 0),
                            stop=(k == KO1 - 1),
                        )
                    relu_drain(pt, m, n)

            # ---- Phase B: obT = W2.T @ hT + b2 ----
            for m in range(MT2):
                ot = o_pool.tile([P, C], F32, tag="ot", name=f"ot_{m}")
                pts = [
                    psB.tile([P, N], F32, tag="psB", name=f"psB_{m}_{n}")
                    for n in range(NT)
                ]
                if m < MT2 - 1:
                    for c in range(KO2 // WCH):
                        w2t = w_pool.tile(
                            [P, WCH, P], F32R, tag="w", name=f"w2t_{m}_{c}"
                        )
                        nc.sync.dma_start(
                            w2t[:], w2s.ap()[m, :, ds(c * WCH, WCH), :]
                        )
                        for k in range(WCH):
                            k2 = c * WCH + k
                            for n in range(NT):
                                nc.tensor.matmul(
                                    pts[n][:],
                                    w2t[:, k, :],
                                    ht[:, k2, ds(n * N, N)],
                                    start=(k2 == 0),
                                    stop=(k2 == KO2 - 1),
                                )
                    for n in range(NT):
                        nc.vector.tensor_scalar_add(
                            out=ot[:, ds(n * N, N)],
                            in0=pts[n][:],
                            scalar1=b2t[:, ds(m, 1)],
                        )
                        nc.sync.dma_start(
                            out.ap()[m, :, ds(n * N, N)], ot[:, ds(n * N, N)]
                        )
                else:
                    # last m-tile: n-outer so the first half's drain and
                    # store overlap the second half's matmuls (shrinks the
                    # kernel tail)
                    w2ts = []
                    for c in range(KO2 // WCH):
                        w2t = w_pool.tile(
                            [P, WCH, P], F32R, tag="w", name=f"w2t_{m}_{c}"
                        )
                        nc.sync.dma_start(
                            w2t[:], w2s.ap()[m, :, ds(c * WCH, WCH), :]
                        )
                        w2ts.append(w2t)
                    for n in range(NT):
                        for c in range(KO2 // WCH):
                            for k in range(WCH):
                                k2 = c * WCH + k
                                nc.tensor.matmul(
                                    pts[n][:],
                                    w2ts[c][:, k, :],
                                    ht[:, k2, ds(n * N, N)],
                                    start=(k2 == 0),
                                    stop=(k2 == KO2 - 1),
                                )
                        nc.vector.tensor_scalar_add(
                            out=ot[:, ds(n * N, N)],
                            in0=pts[n][:],
                            scalar1=b2t[:, ds(m, 1)],
                        )
                        nc.sync.dma_start(
                            out.ap()[m, :, ds(n * N, N)], ot[:, ds(n * N, N)]
                        )

    return nc


_NC_CACHE: list = []


def _get_nc():
    if not _NC_CACHE:
        _NC_CACHE.append(build_expert_ffn())
    return _NC_CACHE[0]


# ---------------------------------------------------------------------------
# Host-side routing / dispatch / combine
# ---------------------------------------------------------------------------


def _router_host(x, Wg):
    """Top-1 routing + losses, mirroring the reference op-for-op on jax CPU so
    routing decisions match it bit-exactly."""
    import jax
    import jax.numpy as jnp

    cpu = jax.devices("cpu")[0]
    with jax.default_device(cpu):
        xf = jnp.asarray(np.asarray(x, dtype=np.float32)).reshape(T, D)
        logits = xf @ jnp.asarray(np.asarray(Wg, dtype=np.float32))
        probs = jax.nn.softmax(logits, axis=-1)
        eidx = jnp.argmax(probs, axis=-1)
        gate = jnp.take_along_axis(probs, eidx[:, None], axis=1)[:, 0]
        oh = jax.nn.one_hot(eidx, E, dtype=jnp.int32)
        pos = jnp.cumsum(oh, axis=0)[jnp.arange(T), eidx] - 1
        keep = pos < CAP
        posc = jnp.clip(pos, 0, CAP - 1)
        frac = jnp.mean(oh.astype(xf.dtype), axis=0)
        pmean = jnp.mean(probs, axis=0)
        aux_loss = E * jnp.sum(frac * pmean)
        z_loss = jnp.mean(jax.scipy.special.logsumexp(logits, axis=-1) ** 2)
        return (
            np.asarray(eidx),
            np.asarray(gate),
            np.asarray(keep),
            np.asarray(posc),
            np.asarray(aux_loss),
            np.asarray(z_loss),
        )


def kernel(x, Wg, W1, b1, W2, b2):
    x = np.asarray(x, dtype=np.float32)
    Wg = np.asarray(Wg, dtype=np.float32)
    W1 = np.asarray(W1, dtype=np.float32)
    b1 = np.asarray(b1, dtype=np.float32)
    W2 = np.asarray(W2, dtype=np.float32)
    b2 = np.asarray(b2, dtype=np.float32)

    eidx, gate, keep, posc, aux_loss, z_loss = _router_host(x, Wg)

    # dispatch: scatter kept tokens into per-expert buffers, transposed
    xf = np.ascontiguousarray(x.reshape(T, D))
    buf = np.zeros((E, CAP, D), dtype=np.float32)
    buf[eidx[keep], posc[keep]] = xf[keep]
    bufT = np.ascontiguousarray(buf.transpose(0, 2, 1))  # [E, D, CAP]

    # weight swizzles -> contiguous per-tile DMA layouts
    w1s = np.ascontiguousarray(
        W1.reshape(E, KO1, P, MT1, P).transpose(0, 3, 2, 1, 4)
    )
    w2s = np.ascontiguousarray(
        W2.reshape(E, KO2, P, MT2, P).transpose(0, 3, 2, 1, 4)
    )
    b1c = np.ascontiguousarray(b1.reshape(E, MT1, P).transpose(0, 2, 1))
    b2c = np.ascontiguousarray(b2.reshape(E, MT2, P).transpose(0, 2, 1))

    in_maps = [
        {
            "bufT": bufT[e],
            "w1s": w1s[e],
            "w2s": w2s[e],
            "b1c": b1c[e],
            "b2c": b2c[e],
        }
        for e in range(E)
    ]
    res = run_bass_kernel_spmd(_get_nc(), in_maps, list(range(E)), trace=False)

    # combine: ob_all[e] is obT = [D, CAP]; token t reads column posc[t]
    ob_all = np.stack([res.results[e]["out"].reshape(D, CAP) for e in range(E)])
    y = ob_all[eidx, :, posc] * (gate * keep.astype(np.float32))[:, None]
    y = np.ascontiguousarray(y.reshape(B, S, D).astype(np.float32))

    return y, np.float32(aux_loss), np.float32(z_loss)
